# revision 24
# baseline (speedup 1.0000x reference)
"""Weighted BCE2D loss kernel for Trainium2 (8 NeuronCores, data-parallel).

Computes, for input p and binary target t of shape (32, 1, 1024, 1024) f32:

    pos = sum(t);  neg = S - pos;  S = p.size
    A = sum_{t=1} ln(p);  B = sum_{t=0} ln(1-p)
    loss = -(neg*A + pos*B) / S**2

which equals the reference
    -mean(w * (t*log(p) + (1-t)*log1p(-p))),  w = where(pos, neg/S, pos/S)
(the -100 log-clamp never fires: p is in [1e-4, 1-1e-4] so log >= -9.3).

Staging: the host casts p to fp16 (saturating at 1 - 2^-11 so 1-p never
collapses to 0) and packs t into the fp16 sign bit (pure bitwise OR — all
arithmetic happens on device):

    s = +p  if t == 0,   s = -p  if t == 1

One fp16 stream (8.4 MB/core) is the whole HBM traffic.

Device: two chunk pipelines, mixed to balance DVE vs ACT vs PE (accum-
carrying DVE tensor_scalars degrade to 1x — measured — so every DVE op
here is accum-free; ACT accumulators are free):

kind "ab" (DVE-light, 2 ACT passes, no PE):
    x2 = min(-s, 0)     tensor_scalar (mult -1, min 0), 4x
    x3 = max(-s, 0)     tensor_scalar (mult -1, max 0), 4x
    Ln(x2 + 1)  accum -> B_dev = B + pos_chunk * ln_dev(1)       [= B]
    Ln(x3 + b)  accum -> A_dev = A + neg_chunk * ln_dev(b)
    (b = 2e-5; ln_dev(1), ln_dev(b) measured on-device by feeding 0
    through the same Ln, so the constant leak cancels exactly)

kind "mul" (classic masked form, B reduced on the PE):
    tinv = (s > 0)      tensor_scalar is_gt, 4x
    q    = tinv - s     tensor_tensor subtract, 2x    (= p or 1-p)
    l    = Ln(q)        accum -> sum(ln q) = A + B
    m    = tinv * l     tensor_tensor mult, 2x
    psum += ones^T @ m  PE matmul chain -> B

both kinds: tinv counted into persistent fp16 accumulators (integer
counts, exact), reduced on the PE at the end.

Host combine per core: A = (Sl_q - B_q) + (Adev - neg_ab*cb),
B = B_q + (Bdev - pos_ab*c1), neg = neg_q + neg_ab, pos = S - neg.

Accuracy: ~1-2e-4 relative error on the loss (numpy- and CoreSim-
verified), far inside the 2e-2 gate.
"""

import sys
import numpy as np

for _p in ("/opt/trn_rl_repo", "/root/.axon_site/_ro/trn_rl_repo"):
    if _p not in sys.path:
        sys.path.append(_p)

N_CORES = 8
N, C, H, W = 32, 1, 1024, 1024
S_TOTAL = N * C * H * W                 # 33_554_432
PER_CORE = S_TOTAL // N_CORES           # 4_194_304
P = 128
WACC = 4096                             # tinv accumulator width
LN_BIAS = 2.0e-5                        # b in the A_dev pass

# (width, index-in-width-view, kind). "ab" = double-Ln pipeline,
# "mul" = masked-multiply pipeline. Short tail chunks shorten the drain.
CHUNKS = [
    (4096, 0, "ab"), (4096, 1, "mul"), (4096, 2, "ab"), (4096, 3, "mul"),
    (4096, 4, "ab"), (4096, 5, "mul"), (2048, 12, "ab"), (2048, 13, "mul"),
    (2048, 14, "tail"), (1024, 30, "tail"), (512, 62, "tail"), (512, 63, "tail"),
]
NCH = len(CHUNKS)
AB_IDX = [i for i, c in enumerate(CHUNKS) if c[2] == "ab"]
MUL_IDX = [i for i, c in enumerate(CHUNKS) if c[2] != "ab"]
S_AB_CORE = sum(CHUNKS[i][0] for i in AB_IDX) * P   # elements in ab-chunks

_CACHE = {}


def _build_program():
    import concourse.bacc as bacc
    import concourse.tile as tile
    from concourse import mybir

    f32 = mybir.dt.float32
    f16 = mybir.dt.float16
    AF = mybir.ActivationFunctionType
    ALU = mybir.AluOpType

    nc = bacc.Bacc("TRN2", target_bir_lowering=False, debug=False,
                   enable_asserts=True, num_devices=N_CORES)

    s_in = nc.dram_tensor("s_in", [PER_CORE], f16, kind="ExternalInput").ap()
    out = nc.dram_tensor("out", [1, 8], f32, kind="ExternalOutput").ap()

    views = {w: s_in.rearrange("(n p f) -> n p f", p=P, f=w)
             for w in sorted({w for w, _, _ in CHUNKS})}

    n_mm = sum(CHUNKS[i][0] // 512 for i in MUL_IDX)
    nQ, nAB = len(MUL_IDX), len(AB_IDX)

    with tile.TileContext(nc) as tc:
        with tc.tile_pool(name="loads", bufs=4) as lpool, \
             tc.tile_pool(name="work", bufs=2) as wpool, \
             tc.tile_pool(name="acc", bufs=1) as apool, \
             tc.tile_pool(name="psum", bufs=1, space="PSUM") as ppool:

            accLq = apool.tile([P, nQ], f32)    # mul-chunks: sum ln(q)
            accBab = apool.tile([P, nAB], f32)  # ab-chunks: B_dev parts
            accAab = apool.tile([P, nAB], f32)  # ab-chunks: A_dev parts
            accTq = apool.tile([P, WACC], f16)  # tinv counts, mul-chunks
            ones_h = apool.tile([P, 1], f16)
            nc.vector.memset(ones_h[:], 1.0)
            bias_b = apool.tile([P, 1], f32)
            nc.vector.memset(bias_b[:], LN_BIAS)
            psum_m = ppool.tile([1, 512], f32)
            psum_t1 = ppool.tile([1, 512], f32)
            psum_t2 = ppool.tile([1, 512], f32)

            mm_i = 0
            t1_mm = 0
            n_t1mm = WACC // 512 + sum(
                CHUNKS[i][0] // 512 for i in range(len(CHUNKS))
                if CHUNKS[i][2] == "tail")
            ab_mm = 0
            n_abmm = sum(CHUNKS[i][0] // 512 for i in AB_IDX)
            qi = ai = 0
            first_q = {True: True}
            first_ab = {True: True}
            for ci, (w, n, kind) in enumerate(CHUNKS):
                src = views[w][n]
                s_t = lpool.tile([P, w], f16, tag="s")
                nc.sync.dma_start(out=s_t[:], in_=src)

                if kind == "ab":
                    x2 = wpool.tile([P, w], f16, tag="q")
                    nc.vector.tensor_scalar(x2[:], s_t[:], -1.0, 0.0,
                                            ALU.mult, ALU.min)
                    x3 = wpool.tile([P, w], f16, tag="l")
                    nc.vector.tensor_scalar(x3[:], s_t[:], -1.0, 0.0,
                                            ALU.mult, ALU.max)
                    nc.scalar.activation(x2[:], x2[:], AF.Ln, bias=1.0,
                                         accum_out=accBab[:, ai:ai + 1])
                    nc.scalar.activation(x3[:], x3[:], AF.Ln,
                                         bias=bias_b[:, 0:1],
                                         accum_out=accAab[:, ai:ai + 1])
                    tinv = wpool.tile([P, w], f16, tag="t")
                    nc.vector.tensor_scalar(tinv[:], s_t[:], 0.0, None,
                                            ALU.is_gt)
                    for j in range(w // 512):
                        sl = slice(j * 512, (j + 1) * 512)
                        nc.tensor.matmul(psum_t2[:], ones_h[:], tinv[:, sl],
                                         start=(ab_mm == 0),
                                         stop=(ab_mm == n_abmm - 1))
                        ab_mm += 1
                    ai += 1
                else:
                    tinv = wpool.tile([P, w], f16, tag="t")
                    nc.vector.tensor_scalar(tinv[:], s_t[:], 0.0, None,
                                            ALU.is_gt)
                    # q = p (t=1) or 1-p (t=0); never 0 (host clamp).
                    q = wpool.tile([P, w], f16, tag="q")
                    nc.vector.tensor_sub(q[:], tinv[:], s_t[:])
                    l = wpool.tile([P, w], f16, tag="l")
                    nc.scalar.activation(l[:], q[:], AF.Ln,
                                         accum_out=accLq[:, qi:qi + 1])
                    if kind == "tail":
                        for j in range(w // 512):
                            sl = slice(j * 512, (j + 1) * 512)
                            nc.tensor.matmul(psum_t1[:], ones_h[:],
                                             tinv[:, sl],
                                             start=False,
                                             stop=(t1_mm == n_t1mm - 1))
                            t1_mm += 1
                    elif first_q[True]:
                        nc.vector.tensor_copy(accTq[:, 0:w], tinv[:])
                        first_q[True] = False
                    else:
                        nc.vector.tensor_add(accTq[:, 0:w], accTq[:, 0:w],
                                             tinv[:])
                    m = wpool.tile([P, w], f16, tag="m")
                    nc.vector.tensor_mul(m[:], tinv[:], l[:])
                    for j in range(w // 512):
                        sl = slice(j * 512, (j + 1) * 512)
                        nc.tensor.matmul(psum_m[:], ones_h[:], m[:, sl],
                                         start=(mm_i == 0),
                                         stop=(mm_i == n_mm - 1))
                        mm_i += 1
                    qi += 1
                    if ci == 7:
                        for j in range(WACC // 512):
                            sl = slice(j * 512, (j + 1) * 512)
                            nc.tensor.matmul(psum_t1[:], ones_h[:],
                                             accTq[:, sl],
                                             start=(j == 0), stop=False)
                            t1_mm += 1


            # Calibration: ln through the same table at the two leak points.
            zer = apool.tile([1, 1], f16)
            nc.vector.memset(zer[:], 0.0)
            calb = apool.tile([1, 1], f32)
            nc.scalar.activation(calb[:], zer[:], AF.Ln,
                                 bias=bias_b[0:1, 0:1])
            cal1 = apool.tile([1, 1], f32)
            nc.scalar.activation(cal1[:], zer[:], AF.Ln, bias=1.0)

            # Epilogue: fold partials into out[1,8]:
            # [Sl_q, neg_q, B_q, Bdev, Adev, neg_ab, cb, c1]
            red = apool.tile([P, 3], f32)
            nc.vector.tensor_reduce(red[:, 0:1], accLq[:],
                                    axis=mybir.AxisListType.X, op=ALU.add)
            nc.vector.tensor_reduce(red[:, 1:2], accBab[:],
                                    axis=mybir.AxisListType.X, op=ALU.add)
            nc.vector.tensor_reduce(red[:, 2:3], accAab[:],
                                    axis=mybir.AxisListType.X, op=ALU.add)
            ones_f = apool.tile([P, 1], f32)
            nc.vector.memset(ones_f[:], 1.0)
            psum_f = ppool.tile([1, 3], f32)
            nc.tensor.matmul(psum_f[:], ones_f[:], red[:],
                             start=True, stop=True)
            res = apool.tile([1, 8], f32)
            nc.vector.memset(res[:], 0.0)
            nc.vector.tensor_copy(res[0:1, 0:1], psum_f[0:1, 0:1])
            nc.vector.tensor_reduce(res[0:1, 1:2], psum_t1[0:1, :],
                                    axis=mybir.AxisListType.X, op=ALU.add)
            nc.vector.tensor_reduce(res[0:1, 2:3], psum_m[0:1, :],
                                    axis=mybir.AxisListType.X, op=ALU.add)
            nc.vector.tensor_copy(res[0:1, 3:4], psum_f[0:1, 1:2])
            nc.vector.tensor_copy(res[0:1, 4:5], psum_f[0:1, 2:3])
            nc.vector.tensor_reduce(res[0:1, 5:6], psum_t2[0:1, :],
                                    axis=mybir.AxisListType.X, op=ALU.add)
            nc.vector.tensor_copy(res[0:1, 6:7], calb[0:1, :])
            nc.vector.tensor_copy(res[0:1, 7:8], cal1[0:1, :])
            nc.sync.dma_start(out=out[0:1, :], in_=res[:])

    nc.compile()
    return nc


def _get_program():
    if "nc" not in _CACHE:
        _CACHE["nc"] = _build_program()
    return _CACHE["nc"]


def pack_inputs(input, target):
    """fp16 saturating cast of p; target bit ORed into the sign bit."""
    p = np.asarray(input, dtype=np.float32).reshape(-1)
    t = np.asarray(target).reshape(-1)
    ph = np.minimum(p, np.float32(1.0 - 2.0 ** -11)).astype(np.float16)
    tbit = (t > 0).astype(np.uint16) << np.uint16(15)
    s = (ph.view(np.uint16) | tbit).view(np.float16)
    return s.reshape(N_CORES, PER_CORE)


def run_on_device(input, target, trace=False, **kw):
    """Shard, run on 8 cores, return (partials [8,8], BassKernelResults)."""
    from concourse import bass_utils

    nc = _get_program()
    s = pack_inputs(input, target)
    in_maps = [{"s_in": s[k]} for k in range(N_CORES)]
    res = bass_utils.run_bass_kernel_spmd(
        nc, in_maps, core_ids=list(range(N_CORES)), trace=trace, **kw)
    partials = np.stack([res.results[k]["out"][0, :] for k in range(N_CORES)])
    return partials, res


def _combine(partials):
    p64 = partials.astype(np.float64)
    Sl_q = p64[:, 0]
    neg_q = p64[:, 1]
    B_q = p64[:, 2]
    Bdev = p64[:, 3]
    Adev = p64[:, 4]
    neg_ab = p64[:, 5]
    cb = p64[:, 6]     # ln_dev(LN_BIAS), same on all cores
    c1 = p64[:, 7]     # ln_dev(1.0), same on all cores
    pos_ab = S_AB_CORE - neg_ab
    A = (Sl_q - B_q) + (Adev - neg_ab * cb)
    B = B_q + (Bdev - pos_ab * c1)
    A, B = float(A.sum()), float(B.sum())
    neg = float((neg_q + neg_ab).sum())
    pos = S_TOTAL - neg
    loss = -(neg * A + pos * B) / (float(S_TOTAL) ** 2)
    return np.asarray(loss, dtype=np.float32)


def kernel(input, target):
    partials, _ = run_on_device(input, target)
    return _combine(partials)
